# revision 44
# baseline (speedup 1.0000x reference)
"""DA3CrossFrameRKDAngleLoss Trainium2 kernel (bass/Tile).  v5 — k-major layout

Sharding: 8 cores = (batch b = core//2) x (ref-row half = core%2).
Each core handles R=128 ref rows of one batch; host sums partial sums.

K-major pick order: pick m = k*128 + r  (r = partition, k = free group).
With that order, every per-(r,s) quantity (T, qsr) is directly broadcastable
along the k free dim — no replication matmuls needed at all.

Per-core math (R=128, S=256, K=4, D=1024, E=4096):
  sim[r,e] = ref_t[r] . extra_unit[e]   bf16 MM, fp32 psum (row scale irrelevant)
  top4 per row -> idx; transposed gather simT[d, m] = extra[idx_m][d] (bf16)
  rh[n][r,k] = diag of refb[n]^T @ simT        (MM + eye-mask diag extract)
  hh[r,k]   = diag of simT_k^T @ simT_k        (Gram + diag extract)
  sr'[r,(f,n,s)] = ref.shared (bf16 MM);  T = -2*sr' + (rr+ss);  qsr = rsqrt(T)
  P1[r,k,(n,s)] = sim.shared - sr'   (bf16 MMs + eye-fold of -sr')
  a1 = (P1*qhr + b1) * qsr_bc;  a2 = (-P1*qhr + b2) * qsh
  a3 = ((T_bc - P1) + b3) * qsr * qsh;  qsh = rsqrt(T_bc - 2*P1 + (hh-rr))
  acc[f,a] = sum |a_teacher - a_student|
loss = sum(acc over all cores) / (3*B*256*256*4)
"""
import sys
sys.path.insert(0, '/opt/trn_rl_repo')
import numpy as np
import ml_dtypes

import concourse.bass as bass
import concourse.mybir as mybir
import concourse.tile as tile
from concourse import bacc
from concourse.bass_utils import run_bass_kernel_spmd

AF = mybir.ActivationFunctionType
OP = mybir.AluOpType
F32 = mybir.dt.float32
BF16 = mybir.dt.bfloat16
F16 = mybir.dt.float16
F8 = mybir.dt.float8e4

R, S, K, D, E = 128, 256, 4, 1024, 4096
RK = R * K
NF = 3
KC = D // 128          # 8
EC = E // 512          # 8


def build_program(n_cores=8):
    nc = bacc.Bacc("TRN2", target_bir_lowering=False, debug=False,
                   num_devices=n_cores)
    d = {}
    d['extra_nt'] = nc.dram_tensor("extra_nt", [D, E], BF16, kind="ExternalInput").ap()
    # pre-shuffled partition-major: [n][p][c*R+r] = ref_n[d=c*128+p][r]
    d['reft_bf'] = nc.dram_tensor("reft_bf", [2, 128, KC * R], BF16, kind="ExternalInput").ap()
    # pre-shuffled partition-major: [f][p][c*2S+s] = shared_f[d=c*128+p][s]
    d['sh_bf'] = nc.dram_tensor("sh_bf", [NF, 128, KC * 2 * S], BF16, kind="ExternalInput").ap()
    d['w_rs'] = nc.dram_tensor("w_rs", [NF, 2, 128, S], F32, kind="ExternalInput").ap()
    d['extra_bf'] = nc.dram_tensor("extra_bf", [E, D + 256], BF16, kind="ExternalInput").ap()
    d['rr'] = nc.dram_tensor("rr", [2, 128, 1], F32, kind="ExternalInput").ap()
    d['eyeb'] = nc.dram_tensor("eyeb", [128, RK], BF16, kind="ExternalInput").ap()
    d['acc'] = nc.dram_tensor("acc", [128, NF * 3], F32, kind="ExternalOutput").ap()
    d['idx'] = nc.dram_tensor("idx", [128, 8], mybir.dt.uint16, kind="ExternalOutput").ap()

    with tile.TileContext(nc) as tc:
        _body(nc, tc, d)
    nc.compile()
    return nc


def _body(nc, tc, d):
    from contextlib import ExitStack
    with ExitStack() as ctx:
        sb = ctx.enter_context(tc.tile_pool(name="persist", bufs=1))

        # ---- resident tiles; refb first on sync queue (sim stationary) ----
        refb = [sb.tile([128, KC, R], BF16, tag=f"refb{n}", name=f"refb{n}") for n in range(2)]
        sh = [sb.tile([128, KC, 2 * S], BF16, tag=f"sh{f}", name=f"sh{f}") for f in range(NF)]
        w_rs = [[sb.tile([128, S], F32, tag=f"w{f}{n}", name=f"w{f}{n}")
                 for n in range(2)] for f in range(NF)]
        rr = [sb.tile([128, 1], F32, tag=f"rr{n}", name=f"rr{n}") for n in range(2)]
        eyeb = sb.tile([128, RK], BF16, tag="eyeb", name="eyeb")
        for n in range(2):
            nc.sync.dma_start(refb[n][:], d['reft_bf'][n]
                              .rearrange("p (c r) -> p c r", c=KC))
        for f in range(NF):
            nc.scalar.dma_start(sh[f][:], d['sh_bf'][f]
                                .rearrange("p (c s) -> p c s", c=KC))
        for f in range(NF):
            for n in range(2):
                nc.scalar.dma_start(w_rs[f][n][:], d['w_rs'][f, n])
        for n in range(2):
            nc.scalar.dma_start(rr[n][:], d['rr'][n])
        nc.scalar.dma_start(eyeb[:], d['eyeb'])

        simT = sb.tile([128, KC + 2, RK], BF16, tag="simT", name="simT")
        T_sb = [[None] * 2 for _ in range(NF)]
        qsr_sb = [[None] * 2 for _ in range(NF)]
        nsrp_bf = [None] * NF
        acc = sb.tile([128, NF * 3], F32, tag="acc", name="acc")
        rh = [sb.tile([128, K], F32, tag=f"rh{n}", name=f"rh{n}") for n in range(2)]
        hh = sb.tile([128, K], F32, tag="hh", name="hh")

        with tc.tile_pool(name="early", bufs=1) as eb:
            sim_sb = eb.tile([128, E], F32, tag="sim_sb", name="sim_sb")

            # ---- phase 1: sim (bf16), full-E psum [128, 8, 512] ----
            with tc.tile_pool(name="ext", bufs=4) as extp, \
                 tc.tile_pool(name="simps", bufs=1, space="PSUM") as simps:
                ps = simps.tile([128, EC, 512], F32, tag="simps", name="simps")
                for kc in range(KC):
                    x = extp.tile([128, E], BF16, tag="ext", name="ext")
                    nc.sync.dma_start(x[:], d['extra_nt'][kc * 128:(kc + 1) * 128, :])
                    for c in range(EC):
                        nc.tensor.matmul(ps[:, c, :], refb[0][:, kc, :],
                                         x[:, c * 512:(c + 1) * 512],
                                         start=(kc == 0), stop=(kc == KC - 1))
                for c in range(EC):
                    nc.scalar.copy(sim_sb[:, c * 512:(c + 1) * 512], ps[:, c, :])

            # ---- phase 3: topk + gather (emitted before phase 2 so the
            #      vector/gpsimd queues run the gather path first) ----
            mxh = eb.tile([128, 2, 8], F32, tag="mxh", name="mxh")
            nc.vector.max(out=mxh[:, 0, :], in_=sim_sb[:, 0:E // 2])
            nc.vector.max(out=mxh[:, 1, :], in_=sim_sb[:, E // 2:E])
            mx = eb.tile([128, 8], F32, tag="mx", name="mx")
            nc.vector.max(out=mx[:], in_=mxh[:].rearrange("p a b -> p (a b)"))
            mi = eb.tile([128, 8], mybir.dt.uint16, tag="mi", name="mi")
            nc.vector.max_index(out=mi[:], in_max=mx[:], in_values=sim_sb[:])
            idx16 = mi[:, 0:K].bitcast(mybir.dt.int16)

            with tc.tile_pool(name="dram", bufs=1, space="DRAM") as drp:
                # 1) contiguous write (r*4+k)  2) DRAM->DRAM shuffle into the
                # gather's wrapped channel order (2B reads, 16B write runs)
                # 3) contiguous replica loads.  Avoids 2B DRAM writes (ECC RMW).
                idx_dram = drp.tile([RK], mybir.dt.int16, name="idx_dram")
                idx_dram2 = drp.tile([RK], mybir.dt.int16, name="idx_dram2")
                nc.scalar.dma_start(idx_dram[:].rearrange("(p a) -> p a", p=128),
                                    idx16)
                nc.sync.dma_start(
                    idx_dram2[:].rearrange("(q jh jl) -> q jh jl", q=16, jh=4, jl=8),
                    idx_dram[:].rearrange("(jl q jh) -> q jh jl", jl=8, q=16, jh=4))
                idxw = eb.tile([128, RK // 16], mybir.dt.int16, tag="idxw", name="idxw")
                wrapped = idx_dram2[:].rearrange("(q j) -> q j", q=16)
                for sg in range(8):
                    eng = (nc.scalar, nc.sync)[sg % 2]
                    eng.dma_start(idxw[16 * sg:16 * (sg + 1), :], wrapped)
                nc.gpsimd.dma_gather(simT[:], d['extra_bf'], idxw[:], RK, RK, D + 256,
                                     transpose=True, queue_num=0)
            nc.sync.dma_start(d['idx'][:], mi[:])

            # ---- phase 2: sr' (frames merged), T, qsr per net ----
            with tc.tile_pool(name="srps", bufs=2, space="PSUM") as srps, \
                 tc.tile_pool(name="p2", bufs=2) as p2:
                for n in range(2):
                    sp3 = srps.tile([128, NF, S], F32, tag="sp3", name="sp3")
                    for f in range(NF):
                        for kc in range(KC):
                            nc.tensor.matmul(sp3[:, f, :], refb[n][:, kc, :],
                                             sh[f][:, kc, n * S:(n + 1) * S],
                                             start=(kc == 0), stop=(kc == KC - 1))
                    for f in range(NF):
                        T_sb[f][n] = sb.tile([128, S], F32, tag=f"T{f}{n}", name=f"T{f}{n}")
                        nc.vector.scalar_tensor_tensor(out=T_sb[f][n][:],
                                                       in0=sp3[:, f, :], scalar=-2.0,
                                                       in1=w_rs[f][n][:],
                                                       op0=OP.mult, op1=OP.add)
                        nsr = p2.tile([128, S], F32, tag="nsr_tmp", name="nsr_tmp")
                        nc.scalar.activation(nsr[:], T_sb[f][n][:], AF.Sqrt, bias=0.0)
                        qtmp = p2.tile([128, S], F32, tag="q_tmp", name="q_tmp")
                        nc.vector.reciprocal_approx_fast(out=qtmp[:], in_=nsr[:])
                        qsr_sb[f][n] = sb.tile([128, S], F16, tag=f"qsr{f}{n}", name=f"qsr{f}{n}")
                        nc.vector.tensor_copy(out=qsr_sb[f][n][:], in_=qtmp[:])
                        if nsrp_bf[f] is None:
                            nsrp_bf[f] = sb.tile([128, 2, S], BF16, tag=f"nsrp{f}", name=f"nsrp{f}")
                        nc.scalar.activation(nsrp_bf[f][:, n, :], sp3[:, f, :],
                                             AF.Copy, scale=-1.0)

        # ---- phase 4a: rh via matmul + eye-mask diag extract;
        #      hh from the gathered hi/lo norm2 columns (diag extract) ----
        hh_p = [sb.tile([128, K], F32, tag=f"hh{p}", name=f"hh{p}") for p in range(2)]
        with tc.tile_pool(name="rhps", bufs=2, space="PSUM") as rhps, \
             tc.tile_pool(name="diagp", bufs=2) as diagp:
            for part in range(2):
                for k in range(K):
                    dump = diagp.tile([128, 128], F32, tag="dump", name="dump")
                    nc.vector.scalar_tensor_tensor(out=dump[:],
                                                   in0=simT[:, KC + part, k * 128:(k + 1) * 128],
                                                   scalar=0.0,
                                                   in1=eyeb[:, k * 128:(k + 1) * 128],
                                                   op0=OP.bypass, op1=OP.mult,
                                                   accum_out=hh_p[part][:, k:k + 1])
            nc.vector.tensor_add(out=hh[:], in0=hh_p[0][:], in1=hh_p[1][:])
            for n in range(2):
                full = rhps.tile([128, RK], F32, tag="full", name="full")
                for kc in range(KC):
                    nc.tensor.matmul(full[:], refb[n][:, kc, :], simT[:, kc, :],
                                     start=(kc == 0), stop=(kc == KC - 1))
                for k in range(K):
                    dump = diagp.tile([128, 128], F32, tag="dump", name="dump")
                    nc.vector.scalar_tensor_tensor(out=dump[:],
                                                   in0=full[:, k * 128:(k + 1) * 128],
                                                   scalar=0.0,
                                                   in1=eyeb[:, k * 128:(k + 1) * 128],
                                                   op0=OP.bypass, op1=OP.mult,
                                                   accum_out=rh[n][:, k:k + 1])

        # ---- phase 4b: per-(r,k) scalars ----
        qhr = [None] * 2; qhrn = [None] * 2
        b1 = [None] * 2; b2 = [None] * 2; b3 = [None] * 2; bshv = [None] * 2
        for n in range(2):
            u1 = sb.tile([128, K], F32, tag=f"u1{n}", name=f"u1{n}")
            nc.vector.scalar_tensor_tensor(out=u1[:], in0=rh[n][:], scalar=-1.0,
                                           in1=rr[n][:].broadcast_to([128, K]),
                                           op0=OP.mult, op1=OP.add)
            u2 = sb.tile([128, K], F32, tag=f"u2{n}", name=f"u2{n}")
            nc.vector.scalar_tensor_tensor(out=u2[:], in0=rh[n][:], scalar=-1.0,
                                           in1=hh[:], op0=OP.mult, op1=OP.add)
            nhr = sb.tile([128, K], F32, tag=f"nhr{n}", name=f"nhr{n}")
            nc.vector.tensor_add(out=nhr[:], in0=u1[:], in1=u2[:])
            nc.scalar.activation(nhr[:], nhr[:], AF.Sqrt, bias=0.0)
            qhr[n] = sb.tile([128, K], F32, tag=f"qhr{n}", name=f"qhr{n}")
            nc.vector.reciprocal_approx_fast(out=qhr[n][:], in_=nhr[:])
            qhrn[n] = sb.tile([128, K], F32, tag=f"qhrn{n}", name=f"qhrn{n}")
            nc.vector.tensor_scalar_mul(qhrn[n][:], qhr[n][:], -1.0)
            b1[n] = sb.tile([128, K], F32, tag=f"b1{n}", name=f"b1{n}")
            nc.vector.tensor_mul(out=b1[n][:], in0=u1[:], in1=qhr[n][:])
            b2[n] = sb.tile([128, K], F32, tag=f"b2{n}", name=f"b2{n}")
            nc.vector.tensor_mul(out=b2[n][:], in0=u2[:], in1=qhr[n][:])
            b3[n] = sb.tile([128, K], F32, tag=f"b3{n}", name=f"b3{n}")
            nc.vector.tensor_scalar_mul(b3[n][:], u1[:], -1.0)
            bshv[n] = sb.tile([128, K], F32, tag=f"bsh{n}", name=f"bsh{n}")
            nc.vector.tensor_sub(out=bshv[n][:], in0=u2[:], in1=u1[:])

        # ---- phases 5-6: per frame (P1 double-buffered) ----
        with tc.tile_pool(name="p1ps", bufs=2, space="PSUM") as p1p, \
             tc.tile_pool(name="ang", bufs=3) as ang, \
             tc.tile_pool(name="angb", bufs=2) as angb:
            for f in range(NF):
                p1 = p1p.tile([128, K, 2 * S], F32, tag="p1", name="p1")
                for k in range(K):
                    nc.tensor.matmul(p1[:, k, :],
                                     eyeb[:, k * 128:(k + 1) * 128],
                                     nsrp_bf[f][:].rearrange("p a b -> p (a b)"),
                                     start=True, stop=False)
                for kc in range(KC):
                    for k in range(K):
                        nc.tensor.matmul(p1[:, k, :],
                                         simT[:, kc, k * 128:(k + 1) * 128],
                                         sh[f][:, kc, :],
                                         start=False, stop=(kc == KC - 1))
                a_out = [[None] * 2 for _ in range(3)]
                p1c = [None] * 2
                for n in range(2):
                    # free the psum early: one copy, everything reads SBUF f16
                    p1c[n] = ang.tile([128, K, S], F16, tag=f"p1c{n}", name=f"p1c{n}")
                    nc.scalar.copy(p1c[n][:], p1[:, :, n * S:(n + 1) * S])
                for n in range(2):
                    p1s = p1c[n][:]
                    T_bc = T_sb[f][n][:].unsqueeze(1).broadcast_to([128, K, S])
                    qsr_bc = qsr_sb[f][n][:].unsqueeze(1).broadcast_to([128, K, S])
                    ta1 = ang.tile([128, K, S], F16, tag="ta1", name="ta1")
                    ta2 = ang.tile([128, K, S], F16, tag="ta2", name="ta2")
                    for k in range(K):
                        nc.scalar.activation(ta1[:, k, :], p1c[n][:, k, :],
                                             AF.Identity, scale=qhr[n][:, k:k + 1],
                                             bias=b1[n][:, k:k + 1])
                        nc.scalar.activation(ta2[:, k, :], p1c[n][:, k, :],
                                             AF.Identity, scale=qhrn[n][:, k:k + 1],
                                             bias=b2[n][:, k:k + 1])
                    t5 = ang.tile([128, K, S], F16, tag="t5", name="t5")
                    nc.vector.scalar_tensor_tensor(out=t5[:], in0=p1s, scalar=-1.0,
                                                   in1=T_bc, op0=OP.mult, op1=OP.add)
                    t6 = ang.tile([128, K, S], F16, tag="t6", name="t6")
                    nc.vector.scalar_tensor_tensor(out=t6[:], in0=p1s, scalar=-2.0,
                                                   in1=T_bc, op0=OP.mult, op1=OP.add)
                    nshf = ang.tile([128, K, S], F32, tag="nshf", name="nshf")
                    for k in range(K):
                        nc.scalar.activation(nshf[:, k, :], t6[:, k, :], AF.Sqrt,
                                             bias=bshv[n][:, k:k + 1])
                    qshf = ang.tile([128, K, S], F32, tag="qshf", name="qshf")
                    nc.vector.reciprocal_approx_fast(out=qshf[:], in_=nshf[:])
                    qsh = ang.tile([128, K, S], F16, tag="qsh", name="qsh")
                    nc.scalar.copy(qsh[:], qshf[:])
                    a1 = angb.tile([128, K, S], F16, tag=f"a1_{n}", name=f"a1_{n}")
                    nc.gpsimd.tensor_mul(out=a1[:], in0=ta1[:], in1=qsr_bc)
                    a2 = angb.tile([128, K, S], F16, tag=f"a2_{n}", name=f"a2_{n}")
                    nc.vector.tensor_mul(out=a2[:], in0=ta2[:], in1=qsh[:])
                    a3q = ang.tile([128, K, S], F16, tag="a3q", name="a3q")
                    for k in range(K):
                        nc.vector.scalar_tensor_tensor(out=a3q[:, k, :], in0=t5[:, k, :],
                                                       scalar=b3[n][:, k:k + 1],
                                                       in1=qsr_sb[f][n][:],
                                                       op0=OP.add, op1=OP.mult)
                    a3 = angb.tile([128, K, S], F16, tag=f"a3_{n}", name=f"a3_{n}")
                    nc.gpsimd.tensor_mul(out=a3[:], in0=a3q[:], in1=qsh[:])
                    a_out[0][n], a_out[1][n], a_out[2][n] = a1, a2, a3
                for a in range(3):
                    dd = angb.tile([128, K, S], F16, tag="dd", name="dd")
                    nc.gpsimd.tensor_sub(out=dd[:], in0=a_out[a][0][:],
                                         in1=a_out[a][1][:])
                    ddump = angb.tile([128, K, S], F16, tag="ddump", name="ddump")
                    nc.scalar.activation(ddump[:], dd[:], AF.Abs,
                                         accum_out=acc[:, 3 * f + a:3 * f + a + 1])
        nc.sync.dma_start(d['acc'][:], acc[:])


# ======================= host side =======================

EXTRA_FRAMES = (1, 3, 5, 7)
SHARED_TEACHER = (2, 4, 6)
SHARED_STUDENT = (1, 2, 3)


def host_prep(teacher_feats, student_feats, ref_perm, shared_perm, n_cores=8):
    t = np.ascontiguousarray(np.asarray(teacher_feats, dtype=np.float32))
    s = np.ascontiguousarray(np.asarray(student_feats, dtype=np.float32))
    rp = np.asarray(ref_perm).astype(np.int64)
    sp = np.asarray(shared_perm).astype(np.int64)
    eye = np.tile(np.eye(128, dtype=np.float32), (1, K)).astype(ml_dtypes.bfloat16)
    in_maps = []
    for c in range(n_cores):
        b, half = c // 2, c % 2
        rows = rp[half * R:(half + 1) * R]
        ref2 = np.stack([t[b, 0][rows], s[b, 0][rows]])           # [2, R, D]
        extra = t[b][list(EXTRA_FRAMES)].reshape(E, D)
        en = extra / np.maximum(np.linalg.norm(extra, axis=1, keepdims=True), 1e-12)
        # augmented gather rows: [extra bf16 | hi(nrm2) x128 | lo(nrm2) x128]
        ebf = extra.astype(ml_dtypes.bfloat16)
        nrm2 = (ebf.astype(np.float64) ** 2).sum(-1).astype(np.float32)
        hi = nrm2.astype(ml_dtypes.bfloat16)
        lo = (nrm2 - hi.astype(np.float32)).astype(ml_dtypes.bfloat16)
        aug = np.zeros((E, D + 256), dtype=ml_dtypes.bfloat16)
        aug[:, :D] = ebf
        aug[:, D:D + 128] = hi[:, None]
        aug[:, D + 128:] = lo[:, None]
        reftr = np.stack([np.ascontiguousarray(ref2[0].T),
                          np.ascontiguousarray(ref2[1].T)])       # [2, D, R]
        rrv = (ref2.astype(np.float64) ** 2).sum(-1).astype(np.float32)
        sh_bf = np.zeros((NF, D, 2 * S), dtype=ml_dtypes.bfloat16)
        w_rs = np.zeros((NF, 2, 128, S), dtype=np.float32)
        for f in range(NF):
            sh_t = t[b, SHARED_TEACHER[f]][sp]
            sh_s = s[b, SHARED_STUDENT[f]][sp]
            sh_bf[f, :, :S] = sh_t.T
            sh_bf[f, :, S:] = sh_s.T
            ss_t = (sh_t.astype(np.float64) ** 2).sum(-1)
            ss_s = (sh_s.astype(np.float64) ** 2).sum(-1)
            w_rs[f, 0] = rrv[0][:, None] + ss_t[None, :]
            w_rs[f, 1] = rrv[1][:, None] + ss_s[None, :]
        in_maps.append(dict(
            extra_nt=np.ascontiguousarray(en.T).astype(ml_dtypes.bfloat16),
            reft_bf=np.ascontiguousarray(
                reftr.reshape(2, KC, 128, R).transpose(0, 2, 1, 3)
                .reshape(2, 128, KC * R)).astype(ml_dtypes.bfloat16),
            sh_bf=np.ascontiguousarray(
                sh_bf.reshape(NF, KC, 128, 2 * S).transpose(0, 2, 1, 3)
                .reshape(NF, 128, KC * 2 * S)),
            w_rs=w_rs,
            extra_bf=aug,
            rr=np.ascontiguousarray(rrv[:, :, None]),
            eyeb=eye,
        ))
    return in_maps


def host_finish(results, B=4):
    total = 0.0
    for r in results:
        total += float(np.asarray(r["acc"], dtype=np.float64).sum())
    denom = NF * B * 256 * S * K
    return np.array(total / denom, dtype=np.float32)


# ======================= self-contained entry =======================

_NC_CACHE = {}


def kernel(teacher_feats, student_feats, ref_perm, shared_perm):
    """Full-input entry: shards across 8 NeuronCores, returns scalar loss (np.float32)."""
    inputs = dict(teacher_feats=np.asarray(teacher_feats),
                  student_feats=np.asarray(student_feats),
                  ref_perm=np.asarray(ref_perm),
                  shared_perm=np.asarray(shared_perm))
    n_cores = 8
    if 'nc' not in _NC_CACHE:
        _NC_CACHE['nc'] = build_program(n_cores=n_cores)
    nc = _NC_CACHE['nc']
    in_maps = host_prep(**inputs, n_cores=n_cores)
    res = run_bass_kernel_spmd(nc, in_maps, core_ids=list(range(n_cores)))
    return host_finish(res.results, B=int(inputs['teacher_feats'].shape[0]))


# revision 45
# speedup vs baseline: 1.0268x; 1.0268x over previous
"""DA3CrossFrameRKDAngleLoss Trainium2 kernel (bass/Tile).  v5 — k-major layout

Sharding: 8 cores = (batch b = core//2) x (ref-row half = core%2).
Each core handles R=128 ref rows of one batch; host sums partial sums.

K-major pick order: pick m = k*128 + r  (r = partition, k = free group).
With that order, every per-(r,s) quantity (T, qsr) is directly broadcastable
along the k free dim — no replication matmuls needed at all.

Per-core math (R=128, S=256, K=4, D=1024, E=4096):
  sim[r,e] = ref_t[r] . extra_unit[e]   bf16 MM, fp32 psum (row scale irrelevant)
  top4 per row -> idx; transposed gather simT[d, m] = extra[idx_m][d] (bf16)
  rh[n][r,k] = diag of refb[n]^T @ simT        (MM + eye-mask diag extract)
  hh[r,k]   = diag of simT_k^T @ simT_k        (Gram + diag extract)
  sr'[r,(f,n,s)] = ref.shared (bf16 MM);  T = -2*sr' + (rr+ss);  qsr = rsqrt(T)
  P1[r,k,(n,s)] = sim.shared - sr'   (bf16 MMs + eye-fold of -sr')
  a1 = (P1*qhr + b1) * qsr_bc;  a2 = (-P1*qhr + b2) * qsh
  a3 = ((T_bc - P1) + b3) * qsr * qsh;  qsh = rsqrt(T_bc - 2*P1 + (hh-rr))
  acc[f,a] = sum |a_teacher - a_student|
loss = sum(acc over all cores) / (3*B*256*256*4)
"""
import sys
sys.path.insert(0, '/opt/trn_rl_repo')
import numpy as np
import ml_dtypes

import concourse.bass as bass
import concourse.mybir as mybir
import concourse.tile as tile
from concourse import bacc
from concourse.bass_utils import run_bass_kernel_spmd

AF = mybir.ActivationFunctionType
OP = mybir.AluOpType
F32 = mybir.dt.float32
BF16 = mybir.dt.bfloat16
F16 = mybir.dt.float16
F8 = mybir.dt.float8e4

R, S, K, D, E = 128, 256, 4, 1024, 4096
RK = R * K
NF = 3
KC = D // 128          # 8
EC = E // 512          # 8


def build_program(n_cores=8):
    nc = bacc.Bacc("TRN2", target_bir_lowering=False, debug=False,
                   num_devices=n_cores)
    d = {}
    d['extra_nt'] = nc.dram_tensor("extra_nt", [D, E], BF16, kind="ExternalInput").ap()
    # pre-shuffled partition-major: [n][p][c*R+r] = ref_n[d=c*128+p][r]
    d['reft_bf'] = nc.dram_tensor("reft_bf", [2, 128, KC * R], BF16, kind="ExternalInput").ap()
    # pre-shuffled partition-major: [f][p][c*2S+s] = shared_f[d=c*128+p][s]
    d['sh_bf'] = nc.dram_tensor("sh_bf", [NF, 128, KC * 2 * S], BF16, kind="ExternalInput").ap()
    d['w_rs'] = nc.dram_tensor("w_rs", [NF, 2, 128, S], F32, kind="ExternalInput").ap()
    d['extra_bf'] = nc.dram_tensor("extra_bf", [E, D + 256], BF16, kind="ExternalInput").ap()
    d['rr'] = nc.dram_tensor("rr", [2, 128, 1], F32, kind="ExternalInput").ap()
    d['eyeb'] = nc.dram_tensor("eyeb", [128, RK], BF16, kind="ExternalInput").ap()
    d['acc'] = nc.dram_tensor("acc", [128, NF * 3], F32, kind="ExternalOutput").ap()
    d['idx'] = nc.dram_tensor("idx", [128, 8], mybir.dt.uint16, kind="ExternalOutput").ap()

    with tile.TileContext(nc) as tc:
        _body(nc, tc, d)
    nc.compile()
    return nc


def _body(nc, tc, d):
    from contextlib import ExitStack
    with ExitStack() as ctx:
        sb = ctx.enter_context(tc.tile_pool(name="persist", bufs=1))

        # ---- resident tiles; refb first on sync queue (sim stationary) ----
        refb = [sb.tile([128, KC, R], BF16, tag=f"refb{n}", name=f"refb{n}") for n in range(2)]
        sh = [sb.tile([128, KC, 2 * S], BF16, tag=f"sh{f}", name=f"sh{f}") for f in range(NF)]
        w_rs = [[sb.tile([128, S], F32, tag=f"w{f}{n}", name=f"w{f}{n}")
                 for n in range(2)] for f in range(NF)]
        rr = [sb.tile([128, 1], F32, tag=f"rr{n}", name=f"rr{n}") for n in range(2)]
        eyeb = sb.tile([128, RK], BF16, tag="eyeb", name="eyeb")
        for n in range(2):
            nc.sync.dma_start(refb[n][:], d['reft_bf'][n]
                              .rearrange("p (c r) -> p c r", c=KC))
        for f in range(NF):
            nc.scalar.dma_start(sh[f][:], d['sh_bf'][f]
                                .rearrange("p (c s) -> p c s", c=KC))
        for f in range(NF):
            for n in range(2):
                nc.scalar.dma_start(w_rs[f][n][:], d['w_rs'][f, n])
        for n in range(2):
            nc.scalar.dma_start(rr[n][:], d['rr'][n])
        nc.scalar.dma_start(eyeb[:], d['eyeb'])

        simT = sb.tile([128, KC + 2, RK], BF16, tag="simT", name="simT")
        T_sb = [[None] * 2 for _ in range(NF)]
        qsr_sb = [[None] * 2 for _ in range(NF)]
        nsrp_bf = [None] * NF
        acc = sb.tile([128, NF * 3], F32, tag="acc", name="acc")
        rh = [sb.tile([128, K], F32, tag=f"rh{n}", name=f"rh{n}") for n in range(2)]
        hh = sb.tile([128, K], F32, tag="hh", name="hh")

        with tc.tile_pool(name="early", bufs=1) as eb:
            sim_sb = eb.tile([128, E], F32, tag="sim_sb", name="sim_sb")

            # ---- phase 1: sim (bf16), full-E psum [128, 8, 512] ----
            with tc.tile_pool(name="ext", bufs=4) as extp, \
                 tc.tile_pool(name="simps", bufs=1, space="PSUM") as simps:
                ps = simps.tile([128, EC, 512], F32, tag="simps", name="simps")
                for kc in range(KC):
                    x = extp.tile([128, E], BF16, tag="ext", name="ext")
                    nc.sync.dma_start(x[:], d['extra_nt'][kc * 128:(kc + 1) * 128, :])
                    for c in range(EC):
                        nc.tensor.matmul(ps[:, c, :], refb[0][:, kc, :],
                                         x[:, c * 512:(c + 1) * 512],
                                         start=(kc == 0), stop=(kc == KC - 1))
                for c in range(EC):
                    nc.scalar.copy(sim_sb[:, c * 512:(c + 1) * 512], ps[:, c, :])

            # ---- phase 3: topk + gather (emitted before phase 2 so the
            #      vector/gpsimd queues run the gather path first) ----
            mxh = eb.tile([128, 2, 8], F32, tag="mxh", name="mxh")
            nc.vector.max(out=mxh[:, 0, :], in_=sim_sb[:, 0:E // 2])
            nc.vector.max(out=mxh[:, 1, :], in_=sim_sb[:, E // 2:E])
            mx = eb.tile([128, 8], F32, tag="mx", name="mx")
            nc.vector.max(out=mx[:], in_=mxh[:].rearrange("p a b -> p (a b)"))
            mi = eb.tile([128, 8], mybir.dt.uint16, tag="mi", name="mi")
            nc.vector.max_index(out=mi[:], in_max=mx[:], in_values=sim_sb[:])
            idx16 = mi[:, 0:K].bitcast(mybir.dt.int16)

            with tc.tile_pool(name="dram", bufs=1, space="DRAM") as drp:
                # 1) contiguous write (r*4+k)  2) DRAM->DRAM shuffle into the
                # gather's wrapped channel order (2B reads, 16B write runs)
                # 3) contiguous replica loads.  Avoids 2B DRAM writes (ECC RMW).
                idx_dram = drp.tile([RK], mybir.dt.int16, name="idx_dram")
                idx_dram2 = drp.tile([RK], mybir.dt.int16, name="idx_dram2")
                nc.scalar.dma_start(idx_dram[:].rearrange("(p a) -> p a", p=128),
                                    idx16)
                nc.sync.dma_start(
                    idx_dram2[:].rearrange("(q jh jl) -> q jh jl", q=16, jh=4, jl=8),
                    idx_dram[:].rearrange("(jl q jh) -> q jh jl", jl=8, q=16, jh=4))
                idxw = eb.tile([128, RK // 16], mybir.dt.int16, tag="idxw", name="idxw")
                wrapped = idx_dram2[:].rearrange("(q j) -> q j", q=16)
                for sg in range(8):
                    eng = (nc.scalar, nc.sync)[sg % 2]
                    eng.dma_start(idxw[16 * sg:16 * (sg + 1), :], wrapped)
                nc.gpsimd.dma_gather(simT[:], d['extra_bf'], idxw[:], RK, RK, D + 256,
                                     transpose=True, queue_num=0)
            nc.sync.dma_start(d['idx'][:], mi[:])

            # ---- phase 2: sr' (frames merged), T, qsr per net ----
            with tc.tile_pool(name="srps", bufs=2, space="PSUM") as srps, \
                 tc.tile_pool(name="p2", bufs=2) as p2:
                for n in range(2):
                    sp3 = srps.tile([128, NF, S], F32, tag="sp3", name="sp3")
                    for f in range(NF):
                        for kc in range(KC):
                            nc.tensor.matmul(sp3[:, f, :], refb[n][:, kc, :],
                                             sh[f][:, kc, n * S:(n + 1) * S],
                                             start=(kc == 0), stop=(kc == KC - 1))
                    for f in range(NF):
                        T_sb[f][n] = sb.tile([128, S], F32, tag=f"T{f}{n}", name=f"T{f}{n}")
                        nc.vector.scalar_tensor_tensor(out=T_sb[f][n][:],
                                                       in0=sp3[:, f, :], scalar=-2.0,
                                                       in1=w_rs[f][n][:],
                                                       op0=OP.mult, op1=OP.add)
                        nsr = p2.tile([128, S], F32, tag="nsr_tmp", name="nsr_tmp")
                        nc.scalar.activation(nsr[:], T_sb[f][n][:], AF.Sqrt, bias=0.0)
                        qtmp = p2.tile([128, S], F32, tag="q_tmp", name="q_tmp")
                        nc.vector.reciprocal_approx_fast(out=qtmp[:], in_=nsr[:])
                        qsr_sb[f][n] = sb.tile([128, S], F16, tag=f"qsr{f}{n}", name=f"qsr{f}{n}")
                        nc.vector.tensor_copy(out=qsr_sb[f][n][:], in_=qtmp[:])
                        if nsrp_bf[f] is None:
                            nsrp_bf[f] = sb.tile([128, 2, S], BF16, tag=f"nsrp{f}", name=f"nsrp{f}")
                        nc.scalar.activation(nsrp_bf[f][:, n, :], sp3[:, f, :],
                                             AF.Copy, scale=-1.0)

        # ---- phase 4a: rh via matmul + eye-mask diag extract;
        #      hh from the gathered hi/lo norm2 columns (diag extract) ----
        hh_p = [sb.tile([128, K], F32, tag=f"hh{p}", name=f"hh{p}") for p in range(2)]
        with tc.tile_pool(name="rhps", bufs=2, space="PSUM") as rhps, \
             tc.tile_pool(name="diagp", bufs=2) as diagp:
            for part in range(2):
                for k in range(K):
                    dump = diagp.tile([128, 128], F32, tag="dump", name="dump")
                    nc.vector.scalar_tensor_tensor(out=dump[:],
                                                   in0=simT[:, KC + part, k * 128:(k + 1) * 128],
                                                   scalar=0.0,
                                                   in1=eyeb[:, k * 128:(k + 1) * 128],
                                                   op0=OP.bypass, op1=OP.mult,
                                                   accum_out=hh_p[part][:, k:k + 1])
            nc.vector.tensor_add(out=hh[:], in0=hh_p[0][:], in1=hh_p[1][:])
            for n in range(2):
                full = rhps.tile([128, RK], F32, tag="full", name="full")
                for kc in range(KC):
                    nc.tensor.matmul(full[:], refb[n][:, kc, :], simT[:, kc, :],
                                     start=(kc == 0), stop=(kc == KC - 1))
                for k in range(K):
                    dump = diagp.tile([128, 128], F32, tag="dump", name="dump")
                    nc.vector.scalar_tensor_tensor(out=dump[:],
                                                   in0=full[:, k * 128:(k + 1) * 128],
                                                   scalar=0.0,
                                                   in1=eyeb[:, k * 128:(k + 1) * 128],
                                                   op0=OP.bypass, op1=OP.mult,
                                                   accum_out=rh[n][:, k:k + 1])

        # ---- phase 4b: per-(r,k) scalars ----
        qhr = [None] * 2; qhrn = [None] * 2
        b1 = [None] * 2; b2 = [None] * 2; b3 = [None] * 2; bshv = [None] * 2
        for n in range(2):
            u1 = sb.tile([128, K], F32, tag=f"u1{n}", name=f"u1{n}")
            nc.vector.scalar_tensor_tensor(out=u1[:], in0=rh[n][:], scalar=-1.0,
                                           in1=rr[n][:].broadcast_to([128, K]),
                                           op0=OP.mult, op1=OP.add)
            u2 = sb.tile([128, K], F32, tag=f"u2{n}", name=f"u2{n}")
            nc.vector.scalar_tensor_tensor(out=u2[:], in0=rh[n][:], scalar=-1.0,
                                           in1=hh[:], op0=OP.mult, op1=OP.add)
            nhr = sb.tile([128, K], F32, tag=f"nhr{n}", name=f"nhr{n}")
            nc.vector.tensor_add(out=nhr[:], in0=u1[:], in1=u2[:])
            nc.scalar.activation(nhr[:], nhr[:], AF.Sqrt, bias=0.0)
            qhr[n] = sb.tile([128, K], F32, tag=f"qhr{n}", name=f"qhr{n}")
            nc.vector.reciprocal_approx_fast(out=qhr[n][:], in_=nhr[:])
            qhrn[n] = sb.tile([128, K], F32, tag=f"qhrn{n}", name=f"qhrn{n}")
            nc.vector.tensor_scalar_mul(qhrn[n][:], qhr[n][:], -1.0)
            b1[n] = sb.tile([128, K], F32, tag=f"b1{n}", name=f"b1{n}")
            nc.vector.tensor_mul(out=b1[n][:], in0=u1[:], in1=qhr[n][:])
            b2[n] = sb.tile([128, K], F32, tag=f"b2{n}", name=f"b2{n}")
            nc.vector.tensor_mul(out=b2[n][:], in0=u2[:], in1=qhr[n][:])
            b3[n] = sb.tile([128, K], F32, tag=f"b3{n}", name=f"b3{n}")
            nc.vector.tensor_scalar_mul(b3[n][:], u1[:], -1.0)
            bshv[n] = sb.tile([128, K], F32, tag=f"bsh{n}", name=f"bsh{n}")
            nc.vector.tensor_sub(out=bshv[n][:], in0=u2[:], in1=u1[:])

        # ---- phases 5-6: per frame (P1 double-buffered) ----
        with tc.tile_pool(name="p1ps", bufs=2, space="PSUM") as p1p, \
             tc.tile_pool(name="ang", bufs=3) as ang, \
             tc.tile_pool(name="angb", bufs=2) as angb:
            for f in range(NF):
                p1 = p1p.tile([128, K, 2 * S], F32, tag="p1", name="p1")
                for k in range(K):
                    nc.tensor.matmul(p1[:, k, :],
                                     eyeb[:, k * 128:(k + 1) * 128],
                                     nsrp_bf[f][:].rearrange("p a b -> p (a b)"),
                                     start=True, stop=False)
                for kc in range(KC):
                    for k in range(K):
                        nc.tensor.matmul(p1[:, k, :],
                                         simT[:, kc, k * 128:(k + 1) * 128],
                                         sh[f][:, kc, :],
                                         start=False, stop=(kc == KC - 1))
                a_out = [[None] * 2 for _ in range(3)]
                p1c = [None] * 2
                for n in range(2):
                    # free the psum early: one copy, everything reads SBUF f16
                    p1c[n] = ang.tile([128, K, S], F16, tag=f"p1c{n}", name=f"p1c{n}")
                    nc.scalar.copy(p1c[n][:], p1[:, :, n * S:(n + 1) * S])
                for n in range(2):
                    p1s = p1c[n][:]
                    T_bc = T_sb[f][n][:].unsqueeze(1).broadcast_to([128, K, S])
                    qsr_bc = qsr_sb[f][n][:].unsqueeze(1).broadcast_to([128, K, S])
                    ta1 = ang.tile([128, K, S], F16, tag="ta1", name="ta1")
                    ta2 = ang.tile([128, K, S], F16, tag="ta2", name="ta2")
                    for k in range(K):
                        nc.scalar.activation(ta1[:, k, :], p1c[n][:, k, :],
                                             AF.Identity, scale=qhr[n][:, k:k + 1],
                                             bias=b1[n][:, k:k + 1])
                        nc.scalar.activation(ta2[:, k, :], p1c[n][:, k, :],
                                             AF.Identity, scale=qhrn[n][:, k:k + 1],
                                             bias=b2[n][:, k:k + 1])
                    t5 = ang.tile([128, K, S], F16, tag="t5", name="t5")
                    nc.vector.scalar_tensor_tensor(out=t5[:], in0=p1s, scalar=-1.0,
                                                   in1=T_bc, op0=OP.mult, op1=OP.add)
                    t6 = ang.tile([128, K, S], F16, tag="t6", name="t6")
                    nc.vector.scalar_tensor_tensor(out=t6[:], in0=p1s, scalar=-2.0,
                                                   in1=T_bc, op0=OP.mult, op1=OP.add)
                    nshf = ang.tile([128, K, S], F32, tag="nshf", name="nshf")
                    for k in range(K):
                        nc.scalar.activation(nshf[:, k, :], t6[:, k, :], AF.Sqrt,
                                             bias=bshv[n][:, k:k + 1])
                    qshf = ang.tile([128, K, S], F32, tag="qshf", name="qshf")
                    nc.vector.reciprocal_approx_fast(out=qshf[:], in_=nshf[:])
                    qsh = ang.tile([128, K, S], F16, tag="qsh", name="qsh")
                    nc.scalar.copy(qsh[:], qshf[:])
                    a1 = angb.tile([128, K, S], F16, tag=f"a1_{n}", name=f"a1_{n}")
                    nc.vector.tensor_mul(out=a1[:], in0=ta1[:], in1=qsr_bc)
                    a2 = angb.tile([128, K, S], F16, tag=f"a2_{n}", name=f"a2_{n}")
                    nc.vector.tensor_mul(out=a2[:], in0=ta2[:], in1=qsh[:])
                    a3q = ang.tile([128, K, S], F16, tag="a3q", name="a3q")
                    for k in range(K):
                        nc.vector.scalar_tensor_tensor(out=a3q[:, k, :], in0=t5[:, k, :],
                                                       scalar=b3[n][:, k:k + 1],
                                                       in1=qsr_sb[f][n][:],
                                                       op0=OP.add, op1=OP.mult)
                    a3 = angb.tile([128, K, S], F16, tag=f"a3_{n}", name=f"a3_{n}")
                    nc.gpsimd.tensor_mul(out=a3[:], in0=a3q[:], in1=qsh[:])
                    a_out[0][n], a_out[1][n], a_out[2][n] = a1, a2, a3
                for a in range(3):
                    dd = angb.tile([128, K, S], F16, tag="dd", name="dd")
                    nc.gpsimd.tensor_sub(out=dd[:], in0=a_out[a][0][:],
                                         in1=a_out[a][1][:])
                    ddump = angb.tile([128, K, S], F16, tag="ddump", name="ddump")
                    nc.scalar.activation(ddump[:], dd[:], AF.Abs,
                                         accum_out=acc[:, 3 * f + a:3 * f + a + 1])
        nc.sync.dma_start(d['acc'][:], acc[:])


# ======================= host side =======================

EXTRA_FRAMES = (1, 3, 5, 7)
SHARED_TEACHER = (2, 4, 6)
SHARED_STUDENT = (1, 2, 3)


def host_prep(teacher_feats, student_feats, ref_perm, shared_perm, n_cores=8):
    t = np.ascontiguousarray(np.asarray(teacher_feats, dtype=np.float32))
    s = np.ascontiguousarray(np.asarray(student_feats, dtype=np.float32))
    rp = np.asarray(ref_perm).astype(np.int64)
    sp = np.asarray(shared_perm).astype(np.int64)
    eye = np.tile(np.eye(128, dtype=np.float32), (1, K)).astype(ml_dtypes.bfloat16)
    in_maps = []
    for c in range(n_cores):
        b, half = c // 2, c % 2
        rows = rp[half * R:(half + 1) * R]
        ref2 = np.stack([t[b, 0][rows], s[b, 0][rows]])           # [2, R, D]
        extra = t[b][list(EXTRA_FRAMES)].reshape(E, D)
        en = extra / np.maximum(np.linalg.norm(extra, axis=1, keepdims=True), 1e-12)
        # augmented gather rows: [extra bf16 | hi(nrm2) x128 | lo(nrm2) x128]
        ebf = extra.astype(ml_dtypes.bfloat16)
        nrm2 = (ebf.astype(np.float64) ** 2).sum(-1).astype(np.float32)
        hi = nrm2.astype(ml_dtypes.bfloat16)
        lo = (nrm2 - hi.astype(np.float32)).astype(ml_dtypes.bfloat16)
        aug = np.zeros((E, D + 256), dtype=ml_dtypes.bfloat16)
        aug[:, :D] = ebf
        aug[:, D:D + 128] = hi[:, None]
        aug[:, D + 128:] = lo[:, None]
        reftr = np.stack([np.ascontiguousarray(ref2[0].T),
                          np.ascontiguousarray(ref2[1].T)])       # [2, D, R]
        rrv = (ref2.astype(np.float64) ** 2).sum(-1).astype(np.float32)
        sh_bf = np.zeros((NF, D, 2 * S), dtype=ml_dtypes.bfloat16)
        w_rs = np.zeros((NF, 2, 128, S), dtype=np.float32)
        for f in range(NF):
            sh_t = t[b, SHARED_TEACHER[f]][sp]
            sh_s = s[b, SHARED_STUDENT[f]][sp]
            sh_bf[f, :, :S] = sh_t.T
            sh_bf[f, :, S:] = sh_s.T
            ss_t = (sh_t.astype(np.float64) ** 2).sum(-1)
            ss_s = (sh_s.astype(np.float64) ** 2).sum(-1)
            w_rs[f, 0] = rrv[0][:, None] + ss_t[None, :]
            w_rs[f, 1] = rrv[1][:, None] + ss_s[None, :]
        in_maps.append(dict(
            extra_nt=np.ascontiguousarray(en.T).astype(ml_dtypes.bfloat16),
            reft_bf=np.ascontiguousarray(
                reftr.reshape(2, KC, 128, R).transpose(0, 2, 1, 3)
                .reshape(2, 128, KC * R)).astype(ml_dtypes.bfloat16),
            sh_bf=np.ascontiguousarray(
                sh_bf.reshape(NF, KC, 128, 2 * S).transpose(0, 2, 1, 3)
                .reshape(NF, 128, KC * 2 * S)),
            w_rs=w_rs,
            extra_bf=aug,
            rr=np.ascontiguousarray(rrv[:, :, None]),
            eyeb=eye,
        ))
    return in_maps


def host_finish(results, B=4):
    total = 0.0
    for r in results:
        total += float(np.asarray(r["acc"], dtype=np.float64).sum())
    denom = NF * B * 256 * S * K
    return np.array(total / denom, dtype=np.float32)


# ======================= self-contained entry =======================

_NC_CACHE = {}


def kernel(teacher_feats, student_feats, ref_perm, shared_perm):
    """Full-input entry: shards across 8 NeuronCores, returns scalar loss (np.float32)."""
    inputs = dict(teacher_feats=np.asarray(teacher_feats),
                  student_feats=np.asarray(student_feats),
                  ref_perm=np.asarray(ref_perm),
                  shared_perm=np.asarray(shared_perm))
    n_cores = 8
    if 'nc' not in _NC_CACHE:
        _NC_CACHE['nc'] = build_program(n_cores=n_cores)
    nc = _NC_CACHE['nc']
    in_maps = host_prep(**inputs, n_cores=n_cores)
    res = run_bass_kernel_spmd(nc, in_maps, core_ids=list(range(n_cores)))
    return host_finish(res.results, B=int(inputs['teacher_feats'].shape[0]))
